# revision 7
# baseline (speedup 1.0000x reference)
# ConvLSTM block (B=4,T=16,H=W=64,Cin=32,Cout=64,K=3) + inference BatchNorm,
# as a Bass/Tile kernel for 8 trn2 NeuronCores.
#
# Sharding: core = b*2 + s  (b in 0..3 = batch sample, s in 0..1 = H-half).
# Each core owns 32 output rows of one sample and runs the full T=16 scan on a
# shrinking halo: at step t it computes h/c on (47-t) rows so that no
# inter-core communication is ever needed. The s=1 half is vertically flipped
# on the host (data + kernel rows) so both halves run the same SPMD program.
#
# Layout: channels on SBUF partitions, pixels on the free dim, rows padded to
# width 66 with zero columns (and one zero row above) so every 3x3 tap becomes
# a single flat pixel offset and SAME-padding comes out of reads of zeroed
# cells.
#
# All per-step conv inputs live in ONE double-buffered mega-tensor HT so the
# shifted-duplicate copies collapse into 2 DMA instructions per span (HWDGE
# descriptor generation serializes DMAs at ~625ns each, so DMA count is a
# first-order cost):
#   HT[.., CX:]  x4 = [x@0; x@1; x@66; x@67]  -> 2 quad-tap matmuls
#   HT[.., CA:]  HA = [h@0; h@66]             -> 3 dual-tap matmuls
#   HT[.., CB:]  HB = [h@1; h@0]              -> 1 dual-tap matmul (view +65)
#   HT[.., CM:]  M  = [x@0; x@130; h@132]     -> 1 mixed matmul (view -65)
# 7 conv matmuls per gate chunk = the 864-row contraction minimum. CM is
# offset +198 from 3*NPIX so the three rows-64:128 dup targets (HA band1, HB
# band1, M band) form one arithmetic stride -> a single 3-way dup DMA.
#
# Gate order is permuted to chunk0=[f;i], chunk1=[o;cc]. The 0.2 hard-sigmoid
# scale is folded into the f/i/o weight columns so gates come out of PSUM
# pre-scaled; then:
#   - [f;i]: one DVE tensor_scalar (add bias, max 0); the min(.,1) is fused
#     into the PP multiply (scalar_tensor_tensor min+mult),
#   - o: same clip-lo on gpsimd; min(.,inv) fused into the h multiply,
#   - cc: one ACT Tanh (bias operand),
#   - c' = f*c + i*T as a cross-partition DVE add (PP lower + upper halves),
#   - BatchNorm folds into the weights/gate-bias (y == hidden state, fp16).
import math
from contextlib import ExitStack

import numpy as np

import concourse.bacc as bacc
import concourse.bass as bass
import concourse.mybir as mybir
import concourse.tile as tile
from concourse import bass_utils
from concourse.ap import AP

AF = mybir.ActivationFunctionType
ALU = mybir.AluOpType
F32 = mybir.dt.float32
F16 = mybir.dt.float16

B, T, H, W = 4, 16, 64, 64
CIN, COUT = 32, 64
FR, FW = 49, 66          # frame rows / padded row width
NPIX = FR * FW           # 3234
NCORES = 8
PTILE = 512              # pixel tile (one PSUM bank of fp32)

PITCH = 4 * NPIX + 198   # HT per-partition width
CX, CA, CB, CM = 0, NPIX, 2 * NPIX, 3 * NPIX + 198
XF = CIN * FR * FW       # xin flat per-step stride


def _build_nc(needs_affine: bool) -> bass.Bass:
    nc = bacc.Bacc("TRN2", target_bir_lowering=False, debug=False)

    xin = nc.dram_tensor("xin", [T, CIN, FR, FW], F16, kind="ExternalInput").ap()
    wxs_d = nc.dram_tensor("wxs", [128, 2, 2, 128], F16, kind="ExternalInput").ap()
    wms_d = nc.dram_tensor("wms", [128, 2, 128], F16, kind="ExternalInput").ap()
    wrs_d = nc.dram_tensor("wrs", [128, 4, 2, 128], F16, kind="ExternalInput").ap()
    bv_d = nc.dram_tensor("bvec", [128, 2], F32, kind="ExternalInput").ap()
    iv_d = nc.dram_tensor("invv", [COUT, 3], F32, kind="ExternalInput").ap()
    yout = nc.dram_tensor("yout", [T, COUT, 32, W], F16, kind="ExternalOutput").ap()

    with tile.TileContext(nc) as tc:
        with ExitStack() as ctx:
            consts = ctx.enter_context(tc.tile_pool(name="consts", bufs=1))
            gpool = ctx.enter_context(tc.tile_pool(name="gpool", bufs=4, space="PSUM"))

            wxs = consts.tile([128, 2, 2, 128], F16, tag="wxs")
            wms = consts.tile([128, 2, 128], F16, tag="wms")
            wrs = consts.tile([128, 4, 2, 128], F16, tag="wrs")
            bv = consts.tile([128, 2], F32, tag="bv")
            iv = consts.tile([COUT, 3], F32, tag="iv")
            nc.sync.dma_start(out=wxs[:], in_=wxs_d)
            nc.sync.dma_start(out=wms[:], in_=wms_d)
            nc.sync.dma_start(out=wrs[:], in_=wrs_d)
            nc.sync.dma_start(out=bv[:], in_=bv_d)
            nc.sync.dma_start(out=iv[:], in_=iv_d)

            HTb = [consts.tile([128, PITCH], F16, tag=f"ht{k}", name=f"ht{k}")
                   for k in range(2)]
            CT = consts.tile([128, NPIX], F16, tag="ct")    # [c ; tanh(cc)]
            IFs = consts.tile([128, NPIX], F16, tag="ifs")  # [f_r ; i_r]
            PP = consts.tile([128, NPIX], F16, tag="pp")    # [f*c ; i*T]
            ost = consts.tile([COUT, NPIX], F16, tag="ost")  # o_r
            thc = consts.tile([COUT, NPIX], F16, tag="thc")  # tanh(c')
            yst = (consts.tile([COUT, NPIX], F16, tag="yst", name="yst")
                   if needs_affine else None)

            for k in range(2):
                nc.vector.memset(HTb[k][:], 0.0)
            nc.vector.memset(CT[:], 0.0)

            for t in range(T):
                rows = 47 - t                      # computed rows this step
                nx = min((rows + 2) * FW, NPIX)    # x pixels needed
                ht = HTb[t % 2]
                ht_next = HTb[(t + 1) % 2]
                # x loads: 2 paired DMAs for x4 (bands {0,1} and {66,67}),
                # 1 for M's x bands {0,130}
                L = nx - 67
                for p in range(2):
                    dst = AP(tensor=ht[:].tensor, offset=64 * PITCH * p,
                             ap=[[32 * PITCH, 2], [PITCH, 32], [1, L]])
                    src = AP(tensor=xin.tensor, offset=t * XF + 66 * p,
                             ap=[[1, 2], [FR * FW, 32], [1, L]])
                    nc.sync.dma_start(out=dst, in_=src)
                L2 = nx - 130
                dst = AP(tensor=ht[:].tensor, offset=CM,
                         ap=[[32 * PITCH, 2], [PITCH, 32], [1, L2]])
                src = AP(tensor=xin.tensor, offset=t * XF,
                         ap=[[130, 2], [FR * FW, 32], [1, L2]])
                nc.sync.dma_start(out=dst, in_=src)

                p_lo = FW + 1                      # first real pixel (row 1, col 1)
                cnt = rows * FW - 2                # through last real pixel
                ntl = math.ceil(cnt / PTILE)
                bsz = math.ceil(cnt / ntl)
                for j in range(ntl):
                    pj = p_lo + j * bsz
                    nt = min(bsz, p_lo + cnt - pj)
                    g0 = gpool.tile([128, PTILE], F32, tag="g0", name="g0")
                    g1 = gpool.tile([128, PTILE], F32, tag="g1", name="g1")
                    for m, g in ((0, g0), (1, g1)):
                        mms = [
                            (wxs[:, 0, m, :], ht[:, pj - 67: pj - 67 + nt]),
                            (wxs[:, 1, m, :], ht[:, pj: pj + nt]),
                            (wms[:, m, :], ht[:, CM + pj - 65: CM + pj - 65 + nt]),
                        ]
                        if t > 0:
                            for slot, d in ((0, -67), (1, -66), (2, -65)):
                                mms.append((wrs[:, slot, m, :],
                                            ht[:, CA + pj + d: CA + pj + d + nt]))
                            mms.append((wrs[:, 3, m, :],
                                        ht[:, CB + pj + 65: CB + pj + 65 + nt]))
                        for k, (lh, rh) in enumerate(mms):
                            nc.tensor.matmul(
                                g[:, 0:nt], lh, rh,
                                start=(k == 0), stop=(k == len(mms) - 1),
                            )
                    sl = slice(pj, pj + nt)
                    # clip-lo for [f;i] and o (bias folded in); tanh for cc
                    nc.vector.tensor_scalar(
                        IFs[:, sl], g0[:, 0:nt], bv[:, 0:1], 0.0,
                        op0=ALU.add, op1=ALU.max,
                    )
                    nc.gpsimd.tensor_scalar(
                        ost[:, sl], g1[0:COUT, 0:nt], bv[0:COUT, 1:2], 0.0,
                        op0=ALU.add, op1=ALU.max,
                    )
                    nc.scalar.activation(
                        CT[COUT:128, sl], g1[COUT:128, 0:nt], AF.Tanh,
                        bias=bv[COUT:128, 1:2],
                    )
                    # SBUF-only tail ops run on 2-tile blocks to amortize
                    # per-instruction overheads.
                    if j % 2 == 1 or j == ntl - 1:
                        b0 = p_lo + (j - (j % 2)) * bsz if j % 2 == 1 else pj
                        bl = slice(b0, pj + nt)
                        # PP = [min(f_r,1)*c ; min(i_r,1)*T]
                        nc.vector.scalar_tensor_tensor(
                            PP[:, bl], IFs[:, bl], 1.0, CT[:, bl],
                            op0=ALU.min, op1=ALU.mult,
                        )
                        # c' = f*c + i*T  (cross-partition add)
                        nc.vector.tensor_tensor(
                            CT[0:COUT, bl], PP[0:COUT, bl], PP[COUT:128, bl],
                            ALU.add,
                        )
                        nc.scalar.activation(thc[:, bl], CT[0:COUT, bl], AF.Tanh)

                # h = min(o_r, inv) * tanh(c') in row-spans so the step tail
                # (h-mult -> dup DMAs -> next step's h matmuls) pipelines with
                # the last tiles' convs instead of serializing behind them.
                ha_next = ht_next[0:COUT, CA:CA + NPIX]

                def _vr(buf, r0, r1):
                    return buf.rearrange("p (r w) -> p r w", w=FW)[
                        :, r0:r1, 1: W + 1
                    ]

                nck = 3
                bounds = [1 + ((rows) * k) // nck for k in range(nck + 1)]
                spans = [(bounds[k], bounds[k + 1],
                          bounds[k] * FW, bounds[k + 1] * FW)
                         for k in range(nck) if bounds[k + 1] > bounds[k]]
                for r0, r1, a, b in spans:
                    nc.gpsimd.scalar_tensor_tensor(
                        _vr(ha_next, r0, r1),
                        _vr(ost[:], r0, r1), iv[:, 0:1], _vr(thc[:], r0, r1),
                        op0=ALU.min, op1=ALU.mult,
                    )
                    if t < T - 1:
                        # 3-way shifted dup (HA band1 @-66, HB band1 @0->+?,
                        # M band @-132 -- arithmetic stride CA+66) + HB band0
                        dsrc = AP(tensor=ht_next[:].tensor, offset=CA + a,
                                  ap=[[PITCH, 64], [0, 3], [1, b - a]])
                        ddst = AP(tensor=ht_next[:].tensor,
                                  offset=64 * PITCH + CA + a - 66,
                                  ap=[[PITCH, 64], [NPIX + 66, 3], [1, b - a]])
                        nc.sync.dma_start(out=ddst, in_=dsrc)
                        nc.sync.dma_start(
                            out=ht_next[0:64, CB + a - 1:CB + b - 1],
                            in_=ht_next[0:64, CA + a:CA + b])
                if needs_affine:
                    nc.scalar.activation(
                        _vr(yst[:], 1, 33), _vr(ha_next, 1, 33),
                        AF.Identity, bias=iv[:, 2:3], scale=iv[:, 1:2],
                    )
                    ysrc = _vr(yst[:], 1, 33)
                else:
                    ysrc = _vr(ha_next, 1, 33)
                nc.sync.dma_start(out=yout[t], in_=ysrc)
    nc.compile()
    return nc


def prepare(x, kernel, rec_kernel, bias, gamma, beta, moving_mean, moving_var):
    """Host-side prep: BN folding, gate permutation, per-core shards."""
    x = np.asarray(x, np.float32)
    kernel = np.asarray(kernel, np.float32)
    rec_kernel = np.asarray(rec_kernel, np.float32)
    bias = np.asarray(bias, np.float32)
    inv = np.asarray(gamma, np.float32) / np.sqrt(
        np.asarray(moving_var, np.float32) + 1e-3
    )
    shift = np.asarray(beta, np.float32) - np.asarray(moving_mean, np.float32) * inv
    fold = bool(np.max(np.abs(shift)) == 0.0)

    # gate blocks in reference order: i,f,cc,o -> chunk0=[f;i], chunk1=[o;cc]
    perm = np.concatenate(
        [np.arange(64, 128), np.arange(0, 64), np.arange(192, 256), np.arange(128, 192)]
    )
    wx_e = kernel[:, :, :, perm]
    wr_e = rec_kernel[:, :, :, perm]
    if fold:
        # state becomes h' = h * inv  (== BN output y); compensate h-conv input
        wr_e = wr_e / inv[None, None, :, None]
    # fold the 0.2 hard-sigmoid input scale (and inv for o) into the weights
    o_iv = inv if fold else np.ones(COUT, np.float32)
    gscale = np.ones(256, np.float32)
    gscale[0:128] = 0.2                     # f, i
    gscale[128:192] = 0.2 * o_iv            # o
    wx_e = wx_e * gscale[None, None, None, :]
    wr_e = wr_e * gscale[None, None, None, :]

    b_p = bias[perm]
    bv0 = 0.5 + 0.2 * b_p[0:128]
    bv1 = np.concatenate([(0.5 + 0.2 * b_p[128:192]) * o_iv, b_p[192:256]])
    bvec = np.stack([bv0, bv1], axis=1).astype(np.float32)
    ivv = np.stack([o_iv, inv, shift], axis=1).astype(np.float32)

    def tap(w, ky, kx, m):  # [Cany, 128] block of gate-chunk m
        return w[ky, kx][:, m * 128:(m + 1) * 128]

    def stack_x(w):  # [3,3,32,256] -> x4 slots [128, 2, 2, 128]
        S = np.zeros((128, 2, 2, 128), np.float32)
        # slot 0 (view -67): bands {0,1,66,67} -> taps (0,0),(0,1),(1,0),(1,1)
        # slot 1 (view 0):   bands -> taps (dup->0),(1,2),(2,1),(2,2)
        tapmap = {(0, 0): (0, 0), (0, 1): (0, 1), (0, 2): (1, 0), (0, 3): (1, 1),
                  (1, 1): (1, 2), (1, 2): (2, 1), (1, 3): (2, 2)}
        for (slot, band), (ky, kx) in tapmap.items():
            for m in (0, 1):
                S[band * 32:(band + 1) * 32, slot, m, :] = tap(w, ky, kx, m)
        return S.astype(np.float16)

    def stack_m(wx, wr):  # M slot: [x@0; x@130; h@132] -> taps (0,2),(2,0),(2,2)
        S = np.zeros((128, 2, 128), np.float32)
        for m in (0, 1):
            S[0:32, m, :] = tap(wx, 0, 2, m)
            S[32:64, m, :] = tap(wx, 2, 0, m)
            S[64:128, m, :] = tap(wr, 2, 2, m)
        return S.astype(np.float16)

    def stack_h(w):  # HA slots 0..2 + HB slot 3 -> [128, 4, 2, 128]
        S = np.zeros((128, 4, 2, 128), np.float32)
        for kx in range(3):   # HA views -67,-66,-65: taps (0,kx),(1,kx)
            for m in (0, 1):
                S[0:64, kx, m, :] = tap(w, 0, kx, m)
                S[64:128, kx, m, :] = tap(w, 1, kx, m)
        for m in (0, 1):      # HB view 65: band0 = h@1 -> (2,1); band1 = h@0 -> (2,0)
            S[0:64, 3, m, :] = tap(w, 2, 1, m)
            S[64:128, 3, m, :] = tap(w, 2, 0, m)
        return S.astype(np.float16)

    in_maps = []
    for core in range(NCORES):
        b, s = core // 2, core % 2
        xs = x[b] if s == 0 else x[b, :, ::-1]
        wx_s = wx_e if s == 0 else wx_e[::-1]
        wr_s = wr_e if s == 0 else wr_e[::-1]
        xf = np.zeros((T, CIN, FR, FW), np.float16)
        xf[:, :, 1:49, 1: W + 1] = xs[:, 0:48].transpose(0, 3, 1, 2)
        in_maps.append(
            dict(
                xin=xf,
                wxs=stack_x(wx_s),
                wms=stack_m(wx_s, wr_s),
                wrs=stack_h(wr_s),
                bvec=bvec,
                invv=ivv,
            )
        )
    return in_maps, fold


def assemble(results):
    y = np.zeros((B, T, H, W, COUT), np.float32)
    for core in range(NCORES):
        b, s = core // 2, core % 2
        blk = results[core]["yout"].astype(np.float32).transpose(0, 2, 3, 1)
        if s == 0:
            y[b, :, 0:32] = blk
        else:
            y[b, :, 32:64] = blk[:, ::-1]
    return y


_NC_CACHE: dict = {}


def get_nc(needs_affine: bool) -> bass.Bass:
    if needs_affine not in _NC_CACHE:
        _NC_CACHE[needs_affine] = _build_nc(needs_affine)
    return _NC_CACHE[needs_affine]


def kernel(**inputs) -> np.ndarray:
    in_maps, fold = prepare(**inputs)
    nc = get_nc(not fold)
    res = bass_utils.run_bass_kernel_spmd(nc, in_maps, core_ids=list(range(NCORES)))
    return assemble(res.results)


# revision 10
# speedup vs baseline: 1.8130x; 1.8130x over previous
# ConvLSTM block (B=4,T=16,H=W=64,Cin=32,Cout=64,K=3) + inference BatchNorm,
# as a Bass/Tile kernel for 8 trn2 NeuronCores.
#
# Sharding: core = b*2 + s  (b in 0..3 = batch sample, s in 0..1 = H-half).
# Each core owns 32 output rows of one sample and runs the full T=16 scan on a
# shrinking halo: at step t it computes h/c on (47-t) rows so that no
# inter-core communication is ever needed. The s=1 half is vertically flipped
# on the host (data + kernel rows) so both halves run the same SPMD program.
#
# Layout: channels on SBUF partitions, pixels on the free dim, rows padded to
# width 66 with zero columns (and one zero row above) so every 3x3 tap becomes
# a single flat pixel offset and SAME-padding comes out of reads of zeroed
# cells.
#
# All per-step conv inputs live in ONE double-buffered mega-tensor HT so the
# shifted-duplicate copies collapse into 2 DMA instructions per span (HWDGE
# descriptor generation serializes DMAs at ~625ns each, so DMA count is a
# first-order cost):
#   HT[.., CX:]  x4 = [x@0; x@1; x@66; x@67]  -> 2 quad-tap matmuls
#   HT[.., CA:]  HA = [h@0; h@66]             -> 3 dual-tap matmuls
#   HT[.., CB:]  HB = [h@1; h@0]              -> 1 dual-tap matmul (view +65)
#   HT[.., CM:]  M  = [x@0; x@130; h@132]     -> 1 mixed matmul (view -65)
# 7 conv matmuls per gate chunk = the 864-row contraction minimum. CM is
# offset +198 from 3*NPIX so the three rows-64:128 dup targets (HA band1, HB
# band1, M band) form one arithmetic stride -> a single 3-way dup DMA.
#
# Gate order is permuted to chunk0=[f;i], chunk1=[o;cc]. The 0.2 hard-sigmoid
# scale is folded into the f/i/o weight columns so gates come out of PSUM
# pre-scaled; then:
#   - [f;i]: one DVE tensor_scalar (add bias, max 0); the min(.,1) is fused
#     into the PP multiply (scalar_tensor_tensor min+mult),
#   - o: same clip-lo on gpsimd; min(.,inv) fused into the h multiply,
#   - cc: one ACT Tanh (bias operand),
#   - c' = f*c + i*T as a cross-partition DVE add (PP lower + upper halves),
#   - BatchNorm folds into the weights/gate-bias (y == hidden state, fp16).
import math
from contextlib import ExitStack

import numpy as np

import concourse.bacc as bacc
import concourse.bass as bass
import concourse.mybir as mybir
import concourse.tile as tile
from concourse import bass_utils
from concourse.ap import AP

AF = mybir.ActivationFunctionType
ALU = mybir.AluOpType
F32 = mybir.dt.float32
F16 = mybir.dt.float16

B, T, H, W = 4, 16, 64, 64
CIN, COUT = 32, 64
FR, FW = 49, 66          # frame rows / padded row width
NPIX = FR * FW           # 3234
NCORES = 8
PTILE = 512              # pixel tile (one PSUM bank of fp32)

PITCH = 4 * NPIX + 198   # HT per-partition width
CX, CA, CB, CM = 0, NPIX, 2 * NPIX, 3 * NPIX + 198
XF = CIN * FR * FW       # xin flat per-step stride


def _build_nc(needs_affine: bool) -> bass.Bass:
    nc = bacc.Bacc("TRN2", target_bir_lowering=False, debug=False)

    xin4 = nc.dram_tensor("xin4", [T, 128, NPIX], F16, kind="ExternalInput").ap()
    xinM = nc.dram_tensor("xinM", [T, 64, NPIX], F16, kind="ExternalInput").ap()
    wxs_d = nc.dram_tensor("wxs", [128, 2, 2, 128], F16, kind="ExternalInput").ap()
    wms_d = nc.dram_tensor("wms", [128, 2, 128], F16, kind="ExternalInput").ap()
    wrs_d = nc.dram_tensor("wrs", [128, 4, 2, 128], F16, kind="ExternalInput").ap()
    bv_d = nc.dram_tensor("bvec", [128, 2], F32, kind="ExternalInput").ap()
    iv_d = nc.dram_tensor("invv", [COUT, 3], F32, kind="ExternalInput").ap()
    yout = nc.dram_tensor("yout", [T, COUT, 32, W], F16, kind="ExternalOutput").ap()

    with tile.TileContext(nc) as tc:
        with ExitStack() as ctx:
            consts = ctx.enter_context(tc.tile_pool(name="consts", bufs=1))
            gpool = ctx.enter_context(tc.tile_pool(name="gpool", bufs=4, space="PSUM"))

            wxs = consts.tile([128, 2, 2, 128], F16, tag="wxs")
            wms = consts.tile([128, 2, 128], F16, tag="wms")
            wrs = consts.tile([128, 4, 2, 128], F16, tag="wrs")
            bv = consts.tile([128, 2], F32, tag="bv")
            iv = consts.tile([COUT, 3], F32, tag="iv")
            nc.sync.dma_start(out=wxs[:], in_=wxs_d)
            nc.sync.dma_start(out=wms[:], in_=wms_d)
            nc.sync.dma_start(out=wrs[:], in_=wrs_d)
            nc.sync.dma_start(out=bv[:], in_=bv_d)
            nc.sync.dma_start(out=iv[:], in_=iv_d)

            HTb = [consts.tile([128, PITCH], F16, tag=f"ht{k}", name=f"ht{k}")
                   for k in range(2)]
            CT = consts.tile([128, NPIX], F16, tag="ct")    # [c ; tanh(cc)]
            IFs = consts.tile([128, NPIX], F16, tag="ifs")  # [f_r ; i_r]
            PP = consts.tile([128, NPIX], F16, tag="pp")    # [f*c ; i*T]
            ost = consts.tile([COUT, NPIX], F16, tag="ost")  # o_r
            thc = consts.tile([COUT, NPIX], F16, tag="thc")  # tanh(c')
            yst = (consts.tile([COUT, NPIX], F16, tag="yst", name="yst")
                   if needs_affine else None)

            for k in range(2):
                nc.vector.memset(HTb[k][:], 0.0)
            nc.vector.memset(CT[:], 0.0)

            for t in range(T):
                rows = 47 - t                      # computed rows this step
                nx = min((rows + 2) * FW, NPIX)    # x pixels needed
                ht = HTb[t % 2]
                ht_next = HTb[(t + 1) % 2]
                # x loads: host pre-stacks the shifted copies, so these are
                # plain rectangular DMAs (HWDGE cost is per-instruction)
                L = nx - 67
                L2 = nx - 130
                nc.sync.dma_start(out=ht[:, 0:L], in_=xin4[t][:, 0:L])
                nc.sync.dma_start(out=ht[0:64, CM:CM + L2], in_=xinM[t][:, 0:L2])

                p_lo = FW + 1                      # first real pixel (row 1, col 1)
                cnt = rows * FW - 2                # through last real pixel
                ntl = math.ceil(cnt / PTILE)
                bsz = math.ceil(cnt / ntl)
                for j in range(ntl):
                    pj = p_lo + j * bsz
                    nt = min(bsz, p_lo + cnt - pj)
                    g0 = gpool.tile([128, PTILE], F32, tag="g0", name="g0")
                    g1 = gpool.tile([128, PTILE], F32, tag="g1", name="g1")
                    for m, g in ((0, g0), (1, g1)):
                        mms = [
                            (wxs[:, 0, m, :], ht[:, pj - 67: pj - 67 + nt]),
                            (wxs[:, 1, m, :], ht[:, pj: pj + nt]),
                            (wms[:, m, :], ht[:, CM + pj - 65: CM + pj - 65 + nt]),
                        ]
                        if t > 0:
                            for slot, d in ((0, -67), (1, -66), (2, -65)):
                                mms.append((wrs[:, slot, m, :],
                                            ht[:, CA + pj + d: CA + pj + d + nt]))
                            mms.append((wrs[:, 3, m, :],
                                        ht[:, CB + pj + 65: CB + pj + 65 + nt]))
                        for k, (lh, rh) in enumerate(mms):
                            nc.tensor.matmul(
                                g[:, 0:nt], lh, rh,
                                start=(k == 0), stop=(k == len(mms) - 1),
                            )
                    sl = slice(pj, pj + nt)
                    # clip-lo for [f;i] and o (bias folded in); tanh for cc
                    nc.vector.tensor_scalar(
                        IFs[:, sl], g0[:, 0:nt], bv[:, 0:1], 0.0,
                        op0=ALU.add, op1=ALU.max,
                    )
                    nc.gpsimd.tensor_scalar(
                        ost[:, sl], g1[0:COUT, 0:nt], bv[0:COUT, 1:2], 0.0,
                        op0=ALU.add, op1=ALU.max,
                    )
                    nc.scalar.activation(
                        CT[COUT:128, sl], g1[COUT:128, 0:nt], AF.Tanh,
                        bias=bv[COUT:128, 1:2],
                    )
                    # SBUF-only tail ops run on 2-tile blocks to amortize
                    # per-instruction overheads.
                    if j % 2 == 1 or j == ntl - 1:
                        b0 = p_lo + (j - (j % 2)) * bsz if j % 2 == 1 else pj
                        bl = slice(b0, pj + nt)
                        # PP = [min(f_r,1)*c ; min(i_r,1)*T]
                        nc.vector.scalar_tensor_tensor(
                            PP[:, bl], IFs[:, bl], 1.0, CT[:, bl],
                            op0=ALU.min, op1=ALU.mult,
                        )
                        # c' = f*c + i*T  (cross-partition add)
                        nc.vector.tensor_tensor(
                            CT[0:COUT, bl], PP[0:COUT, bl], PP[COUT:128, bl],
                            ALU.add,
                        )
                        nc.scalar.activation(thc[:, bl], CT[0:COUT, bl], AF.Tanh)

                # h = min(o_r, inv) * tanh(c') in row-spans so the step tail
                # (h-mult -> dup DMAs -> next step's h matmuls) pipelines with
                # the last tiles' convs instead of serializing behind them.
                ha_next = ht_next[0:COUT, CA:CA + NPIX]

                def _vr(buf, r0, r1):
                    return buf.rearrange("p (r w) -> p r w", w=FW)[
                        :, r0:r1, 1: W + 1
                    ]

                nck = 3
                bounds = [1 + ((rows) * k) // nck for k in range(nck + 1)]
                spans = [(bounds[k], bounds[k + 1],
                          bounds[k] * FW, bounds[k + 1] * FW)
                         for k in range(nck) if bounds[k + 1] > bounds[k]]
                for r0, r1, a, b in spans:
                    nc.gpsimd.scalar_tensor_tensor(
                        _vr(ha_next, r0, r1),
                        _vr(ost[:], r0, r1), iv[:, 0:1], _vr(thc[:], r0, r1),
                        op0=ALU.min, op1=ALU.mult,
                    )
                    if t < T - 1:
                        # 3-way shifted dup (HA band1 @-66, HB band1 @0->+?,
                        # M band @-132 -- arithmetic stride CA+66) + HB band0
                        dsrc = AP(tensor=ht_next[:].tensor, offset=CA + a,
                                  ap=[[PITCH, 64], [0, 3], [1, b - a]])
                        ddst = AP(tensor=ht_next[:].tensor,
                                  offset=64 * PITCH + CA + a - 66,
                                  ap=[[PITCH, 64], [NPIX + 66, 3], [1, b - a]])
                        nc.sync.dma_start(out=ddst, in_=dsrc)
                        nc.sync.dma_start(
                            out=ht_next[0:64, CB + a - 1:CB + b - 1],
                            in_=ht_next[0:64, CA + a:CA + b])
                if needs_affine:
                    nc.scalar.activation(
                        _vr(yst[:], 1, 33), _vr(ha_next, 1, 33),
                        AF.Identity, bias=iv[:, 2:3], scale=iv[:, 1:2],
                    )
                    ysrc = _vr(yst[:], 1, 33)
                else:
                    ysrc = _vr(ha_next, 1, 33)
                nc.sync.dma_start(out=yout[t], in_=ysrc)
    nc.compile()
    return nc


def prepare(x, kernel, rec_kernel, bias, gamma, beta, moving_mean, moving_var):
    """Host-side prep: BN folding, gate permutation, per-core shards."""
    x = np.asarray(x, np.float32)
    kernel = np.asarray(kernel, np.float32)
    rec_kernel = np.asarray(rec_kernel, np.float32)
    bias = np.asarray(bias, np.float32)
    inv = np.asarray(gamma, np.float32) / np.sqrt(
        np.asarray(moving_var, np.float32) + 1e-3
    )
    shift = np.asarray(beta, np.float32) - np.asarray(moving_mean, np.float32) * inv
    fold = bool(np.max(np.abs(shift)) == 0.0)

    # gate blocks in reference order: i,f,cc,o -> chunk0=[f;i], chunk1=[o;cc]
    perm = np.concatenate(
        [np.arange(64, 128), np.arange(0, 64), np.arange(192, 256), np.arange(128, 192)]
    )
    wx_e = kernel[:, :, :, perm]
    wr_e = rec_kernel[:, :, :, perm]
    if fold:
        # state becomes h' = h * inv  (== BN output y); compensate h-conv input
        wr_e = wr_e / inv[None, None, :, None]
    # fold the 0.2 hard-sigmoid input scale (and inv for o) into the weights
    o_iv = inv if fold else np.ones(COUT, np.float32)
    gscale = np.ones(256, np.float32)
    gscale[0:128] = 0.2                     # f, i
    gscale[128:192] = 0.2 * o_iv            # o
    wx_e = wx_e * gscale[None, None, None, :]
    wr_e = wr_e * gscale[None, None, None, :]

    b_p = bias[perm]
    bv0 = 0.5 + 0.2 * b_p[0:128]
    bv1 = np.concatenate([(0.5 + 0.2 * b_p[128:192]) * o_iv, b_p[192:256]])
    bvec = np.stack([bv0, bv1], axis=1).astype(np.float32)
    ivv = np.stack([o_iv, inv, shift], axis=1).astype(np.float32)

    def tap(w, ky, kx, m):  # [Cany, 128] block of gate-chunk m
        return w[ky, kx][:, m * 128:(m + 1) * 128]

    def stack_x(w):  # [3,3,32,256] -> x4 slots [128, 2, 2, 128]
        S = np.zeros((128, 2, 2, 128), np.float32)
        # slot 0 (view -67): bands {0,1,66,67} -> taps (0,0),(0,1),(1,0),(1,1)
        # slot 1 (view 0):   bands -> taps (dup->0),(1,2),(2,1),(2,2)
        tapmap = {(0, 0): (0, 0), (0, 1): (0, 1), (0, 2): (1, 0), (0, 3): (1, 1),
                  (1, 1): (1, 2), (1, 2): (2, 1), (1, 3): (2, 2)}
        for (slot, band), (ky, kx) in tapmap.items():
            for m in (0, 1):
                S[band * 32:(band + 1) * 32, slot, m, :] = tap(w, ky, kx, m)
        return S.astype(np.float16)

    def stack_m(wx, wr):  # M slot: [x@0; x@130; h@132] -> taps (0,2),(2,0),(2,2)
        S = np.zeros((128, 2, 128), np.float32)
        for m in (0, 1):
            S[0:32, m, :] = tap(wx, 0, 2, m)
            S[32:64, m, :] = tap(wx, 2, 0, m)
            S[64:128, m, :] = tap(wr, 2, 2, m)
        return S.astype(np.float16)

    def stack_h(w):  # HA slots 0..2 + HB slot 3 -> [128, 4, 2, 128]
        S = np.zeros((128, 4, 2, 128), np.float32)
        for kx in range(3):   # HA views -67,-66,-65: taps (0,kx),(1,kx)
            for m in (0, 1):
                S[0:64, kx, m, :] = tap(w, 0, kx, m)
                S[64:128, kx, m, :] = tap(w, 1, kx, m)
        for m in (0, 1):      # HB view 65: band0 = h@1 -> (2,1); band1 = h@0 -> (2,0)
            S[0:64, 3, m, :] = tap(w, 2, 1, m)
            S[64:128, 3, m, :] = tap(w, 2, 0, m)
        return S.astype(np.float16)

    in_maps = []
    for core in range(NCORES):
        b, s = core // 2, core % 2
        xs = x[b] if s == 0 else x[b, :, ::-1]
        wx_s = wx_e if s == 0 else wx_e[::-1]
        wr_s = wr_e if s == 0 else wr_e[::-1]
        xf = np.zeros((T, CIN, FR * FW), np.float16)
        xf.reshape(T, CIN, FR, FW)[:, :, 1:49, 1: W + 1] = (
            xs[:, 0:48].transpose(0, 3, 1, 2))
        xin4 = np.zeros((T, 128, NPIX), np.float16)
        for k, sh in enumerate((0, 1, 66, 67)):
            xin4[:, 32 * k:32 * (k + 1), 0:NPIX - sh] = xf[:, :, sh:]
        xinM = np.zeros((T, 64, NPIX), np.float16)
        for k, sh in enumerate((0, 130)):
            xinM[:, 32 * k:32 * (k + 1), 0:NPIX - sh] = xf[:, :, sh:]
        in_maps.append(
            dict(
                xin4=xin4,
                xinM=xinM,
                wxs=stack_x(wx_s),
                wms=stack_m(wx_s, wr_s),
                wrs=stack_h(wr_s),
                bvec=bvec,
                invv=ivv,
            )
        )
    return in_maps, fold


def assemble(results):
    y = np.zeros((B, T, H, W, COUT), np.float32)
    for core in range(NCORES):
        b, s = core // 2, core % 2
        blk = results[core]["yout"].astype(np.float32).transpose(0, 2, 3, 1)
        if s == 0:
            y[b, :, 0:32] = blk
        else:
            y[b, :, 32:64] = blk[:, ::-1]
    return y


_NC_CACHE: dict = {}


def get_nc(needs_affine: bool) -> bass.Bass:
    if needs_affine not in _NC_CACHE:
        _NC_CACHE[needs_affine] = _build_nc(needs_affine)
    return _NC_CACHE[needs_affine]


def kernel(**inputs) -> np.ndarray:
    in_maps, fold = prepare(**inputs)
    nc = get_nc(not fold)
    res = bass_utils.run_bass_kernel_spmd(nc, in_maps, core_ids=list(range(NCORES)))
    return assemble(res.results)


# revision 13
# speedup vs baseline: 1.9004x; 1.0482x over previous
# ConvLSTM block (B=4,T=16,H=W=64,Cin=32,Cout=64,K=3) + inference BatchNorm,
# as a Bass/Tile kernel for 8 trn2 NeuronCores.
#
# Sharding: core = b*2 + s  (b in 0..3 = batch sample, s in 0..1 = H-half).
# Each core owns 32 output rows of one sample and runs the full T=16 scan on a
# shrinking halo: at step t it computes h/c on (47-t) rows so that no
# inter-core communication is ever needed. The s=1 half is vertically flipped
# on the host (data + kernel rows) so both halves run the same SPMD program.
#
# Layout: channels on SBUF partitions, pixels on the free dim, rows padded to
# width 66 with zero columns (and one zero row above) so every 3x3 tap becomes
# a single flat pixel offset and SAME-padding comes out of reads of zeroed
# cells.
#
# All per-step conv inputs live in ONE double-buffered mega-tensor HT so the
# shifted-duplicate copies collapse into 2 DMA instructions per span (HWDGE
# descriptor generation serializes DMAs at ~625ns each, so DMA count is a
# first-order cost):
#   HT[.., CX:]  x4 = [x@0; x@1; x@66; x@67]  -> 2 quad-tap matmuls
#   HT[.., CA:]  HA = [h@0; h@66]             -> 3 dual-tap matmuls
#   HT[.., CB:]  HB = [h@1; h@0]              -> 1 dual-tap matmul (view +65)
#   HT[.., CM:]  M  = [x@0; x@130; h@132]     -> 1 mixed matmul (view -65)
# 7 conv matmuls per gate chunk = the 864-row contraction minimum. CM is
# offset +198 from 3*NPIX so the three rows-64:128 dup targets (HA band1, HB
# band1, M band) form one arithmetic stride -> a single 3-way dup DMA.
#
# Gate order is permuted to chunk0=[f;i], chunk1=[o;cc]. The 0.2 hard-sigmoid
# scale is folded into the f/i/o weight columns so gates come out of PSUM
# pre-scaled; then:
#   - [f;i]: one DVE tensor_scalar (add bias, max 0); the min(.,1) is fused
#     into the PP multiply (scalar_tensor_tensor min+mult),
#   - o: same clip-lo on gpsimd; min(.,inv) fused into the h multiply,
#   - cc: one ACT Tanh (bias operand),
#   - c' = f*c + i*T as a cross-partition DVE add (PP lower + upper halves),
#   - BatchNorm folds into the weights/gate-bias (y == hidden state, fp16).
import math
from contextlib import ExitStack

import numpy as np

import concourse.bacc as bacc
import concourse.bass as bass
import concourse.mybir as mybir
import concourse.tile as tile
from concourse import bass_utils
from concourse.ap import AP

AF = mybir.ActivationFunctionType
ALU = mybir.AluOpType
F32 = mybir.dt.float32
F16 = mybir.dt.float16

B, T, H, W = 4, 16, 64, 64
CIN, COUT = 32, 64
FR, FW = 49, 66          # frame rows / padded row width
NPIX = FR * FW           # 3234
NCORES = 8
PTILE = 512              # pixel tile (one PSUM bank of fp32)

PITCH = 4 * NPIX + 198   # HT per-partition width
CX, CA, CB, CM = 0, NPIX, 2 * NPIX, 3 * NPIX + 198
XF = CIN * FR * FW       # xin flat per-step stride


def _build_nc(needs_affine: bool) -> bass.Bass:
    nc = bacc.Bacc("TRN2", target_bir_lowering=False, debug=False)

    xin4 = nc.dram_tensor("xin4", [T, 128, NPIX], F16, kind="ExternalInput").ap()
    xinM = nc.dram_tensor("xinM", [T, 64, NPIX], F16, kind="ExternalInput").ap()
    wxs_d = nc.dram_tensor("wxs", [128, 2, 2, 128], F16, kind="ExternalInput").ap()
    wms_d = nc.dram_tensor("wms", [128, 2, 128], F16, kind="ExternalInput").ap()
    wrs_d = nc.dram_tensor("wrs", [128, 4, 2, 128], F16, kind="ExternalInput").ap()
    bv_d = nc.dram_tensor("bvec", [128, 2], F32, kind="ExternalInput").ap()
    iv_d = nc.dram_tensor("invv", [COUT, 3], F32, kind="ExternalInput").ap()
    yout = nc.dram_tensor("yout", [T, COUT, 32, W], F16, kind="ExternalOutput").ap()

    with tile.TileContext(nc) as tc:
        with ExitStack() as ctx:
            consts = ctx.enter_context(tc.tile_pool(name="consts", bufs=1))
            gpool = ctx.enter_context(tc.tile_pool(name="gpool", bufs=4, space="PSUM"))

            wxs = consts.tile([128, 2, 2, 128], F16, tag="wxs")
            wms = consts.tile([128, 2, 128], F16, tag="wms")
            wrs = consts.tile([128, 4, 2, 128], F16, tag="wrs")
            bv = consts.tile([128, 2], F32, tag="bv")
            iv = consts.tile([COUT, 3], F32, tag="iv")
            nc.sync.dma_start(out=wxs[:], in_=wxs_d)
            nc.sync.dma_start(out=wms[:], in_=wms_d)
            nc.sync.dma_start(out=wrs[:], in_=wrs_d)
            nc.sync.dma_start(out=bv[:], in_=bv_d)
            nc.sync.dma_start(out=iv[:], in_=iv_d)

            HTb = [consts.tile([128, PITCH], F16, tag=f"ht{k}", name=f"ht{k}")
                   for k in range(2)]
            CT = consts.tile([128, NPIX], F16, tag="ct")    # [c ; tanh(cc)]
            IFs = consts.tile([128, NPIX], F16, tag="ifs")  # [f_r ; i_r]
            PP = consts.tile([128, NPIX], F16, tag="pp")    # [f*c ; i*T]
            ost = consts.tile([COUT, NPIX], F16, tag="ost")  # o_r
            thc = consts.tile([COUT, NPIX], F16, tag="thc")  # tanh(c')
            yst = (consts.tile([COUT, NPIX], F16, tag="yst", name="yst")
                   if needs_affine else None)

            # only the h regions need zeroing (x loads cover their reads);
            # different engines so they run concurrently and don't block the
            # first x load (disjoint columns)
            nc.vector.memset(HTb[0][:, CA:PITCH], 0.0)
            nc.gpsimd.memset(HTb[1][:, CA:PITCH], 0.0)
            nc.vector.memset(CT[:], 0.0)

            for t in range(T):
                rows = 47 - t                      # computed rows this step
                nx = min((rows + 2) * FW, NPIX)    # x pixels needed
                ht = HTb[t % 2]
                ht_next = HTb[(t + 1) % 2]
                # x loads: host pre-stacks the shifted copies, so these are
                # plain rectangular DMAs (HWDGE cost is per-instruction)
                L = nx - 67
                L2 = nx - 130
                nc.sync.dma_start(out=ht[:, 0:L], in_=xin4[t][:, 0:L])
                nc.sync.dma_start(out=ht[0:64, CM:CM + L2], in_=xinM[t][:, 0:L2])

                p_lo = FW + 1                      # first real pixel (row 1, col 1)
                cnt = rows * FW - 2                # through last real pixel
                ntl = math.ceil(cnt / PTILE)
                bsz = math.ceil(cnt / ntl)
                for j in range(ntl):
                    pj = p_lo + j * bsz
                    nt = min(bsz, p_lo + cnt - pj)
                    g0 = gpool.tile([128, PTILE], F32, tag="g0", name="g0")
                    g1 = gpool.tile([128, PTILE], F32, tag="g1", name="g1")
                    for m, g in ((0, g0), (1, g1)):
                        mms = [
                            (wxs[:, 0, m, :], ht[:, pj - 67: pj - 67 + nt]),
                            (wxs[:, 1, m, :], ht[:, pj: pj + nt]),
                            (wms[:, m, :], ht[:, CM + pj - 65: CM + pj - 65 + nt]),
                        ]
                        if t > 0:
                            for slot, d in ((0, -67), (1, -66), (2, -65)):
                                mms.append((wrs[:, slot, m, :],
                                            ht[:, CA + pj + d: CA + pj + d + nt]))
                            mms.append((wrs[:, 3, m, :],
                                        ht[:, CB + pj + 65: CB + pj + 65 + nt]))
                        for k, (lh, rh) in enumerate(mms):
                            nc.tensor.matmul(
                                g[:, 0:nt], lh, rh,
                                start=(k == 0), stop=(k == len(mms) - 1),
                            )
                    sl = slice(pj, pj + nt)
                    # clip-lo for [f;i] and o (bias folded in); tanh for cc
                    nc.vector.tensor_scalar(
                        IFs[:, sl], g0[:, 0:nt], bv[:, 0:1], 0.0,
                        op0=ALU.add, op1=ALU.max,
                    )
                    nc.gpsimd.tensor_scalar(
                        ost[:, sl], g1[0:COUT, 0:nt], bv[0:COUT, 1:2], 0.0,
                        op0=ALU.add, op1=ALU.max,
                    )
                    nc.scalar.activation(
                        CT[COUT:128, sl], g1[COUT:128, 0:nt], AF.Tanh,
                        bias=bv[COUT:128, 1:2],
                    )
                    # SBUF-only tail ops run on 2-tile blocks to amortize
                    # per-instruction overheads.
                    if j % 2 == 1 or j == ntl - 1:
                        b0 = p_lo + (j - (j % 2)) * bsz if j % 2 == 1 else pj
                        bl = slice(b0, pj + nt)
                        # PP = [min(f_r,1)*c ; min(i_r,1)*T]
                        nc.vector.scalar_tensor_tensor(
                            PP[:, bl], IFs[:, bl], 1.0, CT[:, bl],
                            op0=ALU.min, op1=ALU.mult,
                        )
                        # c' = f*c + i*T  (cross-partition add)
                        nc.vector.tensor_tensor(
                            CT[0:COUT, bl], PP[0:COUT, bl], PP[COUT:128, bl],
                            ALU.add,
                        )
                        nc.scalar.activation(thc[:, bl], CT[0:COUT, bl], AF.Tanh)

                # h = min(o_r, inv) * tanh(c') in row-spans so the step tail
                # (h-mult -> dup DMAs -> next step's h matmuls) pipelines with
                # the last tiles' convs instead of serializing behind them.
                ha_next = ht_next[0:COUT, CA:CA + NPIX]

                def _vr(buf, r0, r1):
                    return buf.rearrange("p (r w) -> p r w", w=FW)[
                        :, r0:r1, 1: W + 1
                    ]

                nck = 3
                bounds = [1 + ((rows) * k) // nck for k in range(nck + 1)]
                spans = [(bounds[k], bounds[k + 1],
                          bounds[k] * FW, bounds[k + 1] * FW)
                         for k in range(nck) if bounds[k + 1] > bounds[k]]
                for r0, r1, a, b in spans:
                    nc.gpsimd.scalar_tensor_tensor(
                        _vr(ha_next, r0, r1),
                        _vr(ost[:], r0, r1), iv[:, 0:1], _vr(thc[:], r0, r1),
                        op0=ALU.min, op1=ALU.mult,
                    )
                    if t < T - 1:
                        # 3-way shifted dup (HA band1 @-66, HB band1 @0->+?,
                        # M band @-132 -- arithmetic stride CA+66) + HB band0
                        dsrc = AP(tensor=ht_next[:].tensor, offset=CA + a,
                                  ap=[[PITCH, 64], [0, 3], [1, b - a]])
                        ddst = AP(tensor=ht_next[:].tensor,
                                  offset=64 * PITCH + CA + a - 66,
                                  ap=[[PITCH, 64], [NPIX + 66, 3], [1, b - a]])
                        nc.sync.dma_start(out=ddst, in_=dsrc)
                        nc.sync.dma_start(
                            out=ht_next[0:64, CB + a - 1:CB + b - 1],
                            in_=ht_next[0:64, CA + a:CA + b])
                if needs_affine:
                    nc.scalar.activation(
                        _vr(yst[:], 1, 33), _vr(ha_next, 1, 33),
                        AF.Identity, bias=iv[:, 2:3], scale=iv[:, 1:2],
                    )
                    ybuf = yst
                    yb_off = 0
                else:
                    ybuf = None
                # last step: per-span y DMAs so the final drain overlaps the
                # tail; otherwise one DMA per step (HWDGE is per-instruction)
                yspans = ([(r0, min(33, r1)) for r0, r1, _, _ in spans
                           if r0 < 33] if t == T - 1 else [(1, 33)])
                for y0, y1 in yspans:
                    ysrc = (_vr(yst[:], y0, y1) if needs_affine
                            else _vr(ha_next, y0, y1))
                    nc.sync.dma_start(out=yout[t, :, y0 - 1:y1 - 1, :],
                                      in_=ysrc)
    nc.compile()
    return nc


def prepare(x, kernel, rec_kernel, bias, gamma, beta, moving_mean, moving_var):
    """Host-side prep: BN folding, gate permutation, per-core shards."""
    x = np.asarray(x, np.float32)
    kernel = np.asarray(kernel, np.float32)
    rec_kernel = np.asarray(rec_kernel, np.float32)
    bias = np.asarray(bias, np.float32)
    inv = np.asarray(gamma, np.float32) / np.sqrt(
        np.asarray(moving_var, np.float32) + 1e-3
    )
    shift = np.asarray(beta, np.float32) - np.asarray(moving_mean, np.float32) * inv
    fold = bool(np.max(np.abs(shift)) == 0.0)

    # gate blocks in reference order: i,f,cc,o -> chunk0=[f;i], chunk1=[o;cc]
    perm = np.concatenate(
        [np.arange(64, 128), np.arange(0, 64), np.arange(192, 256), np.arange(128, 192)]
    )
    wx_e = kernel[:, :, :, perm]
    wr_e = rec_kernel[:, :, :, perm]
    if fold:
        # state becomes h' = h * inv  (== BN output y); compensate h-conv input
        wr_e = wr_e / inv[None, None, :, None]
    # fold the 0.2 hard-sigmoid input scale (and inv for o) into the weights
    o_iv = inv if fold else np.ones(COUT, np.float32)
    gscale = np.ones(256, np.float32)
    gscale[0:128] = 0.2                     # f, i
    gscale[128:192] = 0.2 * o_iv            # o
    wx_e = wx_e * gscale[None, None, None, :]
    wr_e = wr_e * gscale[None, None, None, :]

    b_p = bias[perm]
    bv0 = 0.5 + 0.2 * b_p[0:128]
    bv1 = np.concatenate([(0.5 + 0.2 * b_p[128:192]) * o_iv, b_p[192:256]])
    bvec = np.stack([bv0, bv1], axis=1).astype(np.float32)
    ivv = np.stack([o_iv, inv, shift], axis=1).astype(np.float32)

    def tap(w, ky, kx, m):  # [Cany, 128] block of gate-chunk m
        return w[ky, kx][:, m * 128:(m + 1) * 128]

    def stack_x(w):  # [3,3,32,256] -> x4 slots [128, 2, 2, 128]
        S = np.zeros((128, 2, 2, 128), np.float32)
        # slot 0 (view -67): bands {0,1,66,67} -> taps (0,0),(0,1),(1,0),(1,1)
        # slot 1 (view 0):   bands -> taps (dup->0),(1,2),(2,1),(2,2)
        tapmap = {(0, 0): (0, 0), (0, 1): (0, 1), (0, 2): (1, 0), (0, 3): (1, 1),
                  (1, 1): (1, 2), (1, 2): (2, 1), (1, 3): (2, 2)}
        for (slot, band), (ky, kx) in tapmap.items():
            for m in (0, 1):
                S[band * 32:(band + 1) * 32, slot, m, :] = tap(w, ky, kx, m)
        return S.astype(np.float16)

    def stack_m(wx, wr):  # M slot: [x@0; x@130; h@132] -> taps (0,2),(2,0),(2,2)
        S = np.zeros((128, 2, 128), np.float32)
        for m in (0, 1):
            S[0:32, m, :] = tap(wx, 0, 2, m)
            S[32:64, m, :] = tap(wx, 2, 0, m)
            S[64:128, m, :] = tap(wr, 2, 2, m)
        return S.astype(np.float16)

    def stack_h(w):  # HA slots 0..2 + HB slot 3 -> [128, 4, 2, 128]
        S = np.zeros((128, 4, 2, 128), np.float32)
        for kx in range(3):   # HA views -67,-66,-65: taps (0,kx),(1,kx)
            for m in (0, 1):
                S[0:64, kx, m, :] = tap(w, 0, kx, m)
                S[64:128, kx, m, :] = tap(w, 1, kx, m)
        for m in (0, 1):      # HB view 65: band0 = h@1 -> (2,1); band1 = h@0 -> (2,0)
            S[0:64, 3, m, :] = tap(w, 2, 1, m)
            S[64:128, 3, m, :] = tap(w, 2, 0, m)
        return S.astype(np.float16)

    in_maps = []
    for core in range(NCORES):
        b, s = core // 2, core % 2
        xs = x[b] if s == 0 else x[b, :, ::-1]
        wx_s = wx_e if s == 0 else wx_e[::-1]
        wr_s = wr_e if s == 0 else wr_e[::-1]
        xf = np.zeros((T, CIN, FR * FW), np.float16)
        xf.reshape(T, CIN, FR, FW)[:, :, 1:49, 1: W + 1] = (
            xs[:, 0:48].transpose(0, 3, 1, 2))
        xin4 = np.zeros((T, 128, NPIX), np.float16)
        for k, sh in enumerate((0, 1, 66, 67)):
            xin4[:, 32 * k:32 * (k + 1), 0:NPIX - sh] = xf[:, :, sh:]
        xinM = np.zeros((T, 64, NPIX), np.float16)
        for k, sh in enumerate((0, 130)):
            xinM[:, 32 * k:32 * (k + 1), 0:NPIX - sh] = xf[:, :, sh:]
        in_maps.append(
            dict(
                xin4=xin4,
                xinM=xinM,
                wxs=stack_x(wx_s),
                wms=stack_m(wx_s, wr_s),
                wrs=stack_h(wr_s),
                bvec=bvec,
                invv=ivv,
            )
        )
    return in_maps, fold


def assemble(results):
    y = np.zeros((B, T, H, W, COUT), np.float32)
    for core in range(NCORES):
        b, s = core // 2, core % 2
        blk = results[core]["yout"].astype(np.float32).transpose(0, 2, 3, 1)
        if s == 0:
            y[b, :, 0:32] = blk
        else:
            y[b, :, 32:64] = blk[:, ::-1]
    return y


_NC_CACHE: dict = {}


def get_nc(needs_affine: bool) -> bass.Bass:
    if needs_affine not in _NC_CACHE:
        _NC_CACHE[needs_affine] = _build_nc(needs_affine)
    return _NC_CACHE[needs_affine]


def kernel(**inputs) -> np.ndarray:
    in_maps, fold = prepare(**inputs)
    nc = get_nc(not fold)
    res = bass_utils.run_bass_kernel_spmd(nc, in_maps, core_ids=list(range(NCORES)))
    return assemble(res.results)
